# revision 1
# baseline (speedup 1.0000x reference)
"""ChildSumTreeLSTM (complete binary tree, L=16384 leaves, mem=128) on 8 NeuronCores.

Sharding: 8 subtrees of 2048 leaves, data-parallel per level. Each core runs the
same Bass/Tile program on its shard, computing its subtree's 12 levels
(2048 -> 1 nodes). The final two tree levels (4 + 2 nodes) are finished on host.

Layout: feature-major (mem dim on SBUF partitions, nodes along free dim) so all
matmuls need no on-device transposes and the attention bias (h_ext @ W2 + b) is
a per-partition ACT bias fused into the tanh.

Numerics: sigmoid(x) = 0.5 + 0.5*tanh(x/2) folded into pre-scaled weights, so
the whole kernel uses only {tanh, exp} (one ACT table set, loaded once) plus a
DVE reciprocal approximation for the softmax normalization.
"""

import numpy as np

try:
    import concourse.bass as bass
except ImportError:
    import sys

    for p in ("/opt/trn_rl_repo", "/root/.axon_site/_ro/trn_rl_repo"):
        if p not in sys.path:
            sys.path.insert(0, p)
    import concourse.bass as bass

import concourse.bacc as bacc
import concourse.mybir as mybir
import concourse.tile as tile
from concourse import bass_utils

F32 = mybir.dt.float32
AF = mybir.ActivationFunctionType
OP = mybir.AluOpType

L = 16384
MEM = 128
CORES = 8
KDVE = 7  # m-streams routed via DVE exp-path at big levels
LEAF = L // CORES  # 2048 leaves per core

_CACHE = {}


def _build(leaf=LEAF, leaf_chunk=512, int_chunk=512):
    """Build + compile the per-core Bass program. Returns (nc, levels)."""
    levels = []
    n = leaf
    while n >= 4 and n >= leaf // 512:
        levels.append(n)
        n //= 2

    nc = bacc.Bacc("TRN2", debug=False)

    xT = nc.dram_tensor("xT", [128, leaf], F32, kind="ExternalInput")
    Wx = nc.dram_tensor("Wx", [128, 384], F32, kind="ExternalInput")
    Wh = nc.dram_tensor("Wh", [128, 384], F32, kind="ExternalInput")
    Wf = nc.dram_tensor("Wf", [128, 128], F32, kind="ExternalInput")
    W1 = nc.dram_tensor("W1", [128, 128], F32, kind="ExternalInput")
    GB = nc.dram_tensor("GB", [128, 16], F32, kind="ExternalInput")
    WAOH = nc.dram_tensor("WAOH", [128, 256], F32, kind="ExternalInput")
    ONES = nc.dram_tensor("ONES", [16, 16], F32, kind="ExternalInput")
    HEXT = nc.dram_tensor("HEXT", [16, 128], F32, kind="ExternalInput")
    BV = nc.dram_tensor("BV", [128, 8], F32, kind="ExternalInput")
    F16 = nc.dram_tensor("F16", [128, 16], F32, kind="ExternalInput")
    SV = nc.dram_tensor("SV", [16, 2], F32, kind="ExternalInput")
    houts = [
        nc.dram_tensor(f"h{i}", [128, nl], F32, kind="ExternalOutput")
        for i, nl in enumerate(levels)
    ]
    clast = nc.dram_tensor("c_last", [128, levels[-1]], F32, kind="ExternalOutput")

    with tile.TileContext(nc) as tc:
        with (
            tc.tile_pool(name="const", bufs=1) as cp,
            tc.tile_pool(name="state", bufs=1) as st,
            tc.tile_pool(name="work", bufs=3) as wk,
            tc.tile_pool(name="xin", bufs=3) as xp,
            tc.tile_pool(name="psum", bufs=1, space="PSUM") as pp,
        ):
            # ---- constants into SBUF ----
            wx_sb = cp.tile([128, 384], F32)
            wh_sb = cp.tile([128, 384], F32)
            wf_sb = cp.tile([128, 128], F32)
            w1_sb = cp.tile([128, 128], F32)
            gb_sb = cp.tile([128, 16], F32)
            waoh_sb = cp.tile([128, 256], F32)
            ones_sb = cp.tile([16, 16], F32)
            hext_sb = cp.tile([16, 128], F32)
            bv_sb = cp.tile([128, 8], F32)
            f16_sb = cp.tile([128, 16], F32)
            sv_sb = cp.tile([16, 2], F32)
            for sb_t, dr in (
                (wx_sb, Wx), (wh_sb, Wh), (wf_sb, Wf), (w1_sb, W1),
                (gb_sb, GB), (waoh_sb, WAOH), (ones_sb, ONES),
                (hext_sb, HEXT), (bv_sb, BV), (f16_sb, F16), (sv_sb, SV),
            ):
                nc.sync.dma_start(sb_t[:], dr.ap())

            # per-level persistent state: h (true post-attention h) and c2 = 2*c
            h_st = [st.tile([128, nl], F32, name=f"hst{i}", tag=f"hst{i}")
                    for i, nl in enumerate(levels)]
            c2_st = [st.tile([128, nl], F32, name=f"cst{i}", tag=f"cst{i}")
                     for i, nl in enumerate(levels)]

            def attention(hh2, m, lvl, c0):
                """hh2 = 2*h_pre [128, m]; writes h_st[lvl][:, c0:c0+m] and DMAs out."""
                psH = pp.tile([128, m], F32, name="psH", tag="H",
                              padded_shape=[128, leaf_chunk])
                nc.tensor.matmul(psH[:], w1_sb[:], hh2, start=True, stop=True)
                psZ = pp.tile([16, m], F32, name="psZ", tag="Z",
                              padded_shape=[16, leaf_chunk])
                if 16 * m <= 512:
                    # deep levels: build all 16 tanh args with one broadcast
                    # add + one tanh, instead of 16 separate ACT ops
                    targ = wk.tile([128, 16 * m], F32, name="targ", tag="targ",
                                   padded_shape=[128, 512])
                    t3 = targ[:].rearrange("p (g j) -> p g j", g=16)
                    nc.vector.tensor_tensor(
                        t3,
                        psH[:].unsqueeze(1).to_broadcast((128, 16, m)),
                        gb_sb[:].unsqueeze(2).to_broadcast((128, 16, m)),
                        OP.add,
                    )
                    tall = wk.tile([128, 16 * m], F32, name="tall", tag="tall",
                                   padded_shape=[128, 512])
                    nc.scalar.activation(tall[:], targ[:], AF.Tanh)
                    for mm in range(16):
                        nc.tensor.matmul(psZ[:], waoh_sb[:, 16 * mm:16 * mm + 16],
                                         tall[:, mm * m:(mm + 1) * m],
                                         start=(mm == 0), stop=(mm == 15))
                else:
                    # split the 16 m-streams between ACT (tanh) and DVE
                    # (1/(e^{2ha}e^{2g}+1) via recip approx); the -2x+Wsum
                    # correction for DVE rows is folded into the softmax exp
                    # below via per-row scale/bias vectors.
                    ebuf = wk.tile([128, m], F32, name="ebuf", tag="ebuf",
                                   padded_shape=[128, leaf_chunk])
                    nc.scalar.activation(ebuf[:], psH[:], AF.Exp, scale=2.0)
                    for mm in range(16):
                        if mm < 16 - KDVE:
                            tm = wk.tile([128, m], F32, name="tm", tag="tm",
                                         bufs=3, padded_shape=[128, leaf_chunk])
                            nc.scalar.activation(tm[:], psH[:], AF.Tanh,
                                                 bias=gb_sb[:, mm:mm + 1])
                            rhs = tm
                        else:
                            gm = wk.tile([128, m], F32, name="gm", tag="gm",
                                         bufs=3, padded_shape=[128, leaf_chunk])
                            nc.vector.tensor_scalar(
                                gm[:], ebuf[:], f16_sb[:, mm:mm + 1], 1.0,
                                OP.mult, OP.add)
                            rm = wk.tile([128, m], F32, name="rm", tag="rm",
                                         bufs=3, padded_shape=[128, leaf_chunk])
                            nc.vector.reciprocal_approx_fast(rm[:], gm[:])
                            rhs = rm
                        nc.tensor.matmul(psZ[:], waoh_sb[:, 16 * mm:16 * mm + 16],
                                         rhs[:], start=(mm == 0), stop=(mm == 15))
                e16 = wk.tile([16, m], F32, name="e16", tag="e16",
                              padded_shape=[16, leaf_chunk])
                if 16 * m <= 512:
                    nc.scalar.activation(e16[:], psZ[:], AF.Exp)
                else:
                    nc.scalar.activation(e16[:], psZ[:], AF.Exp,
                                         bias=sv_sb[:, 1:2], scale=sv_sb[:, 0:1])
                psS = pp.tile([16, m], F32, name="psS", tag="S",
                              padded_shape=[16, leaf_chunk])
                nc.tensor.matmul(psS[:], ones_sb[:], e16[:], start=True, stop=True)
                r16 = wk.tile([16, m], F32, name="r16", tag="r16",
                              padded_shape=[16, leaf_chunk])
                nc.vector.reciprocal_approx_fast(r16[:], psS[:])
                en = wk.tile([16, m], F32, name="en", tag="en",
                             padded_shape=[16, leaf_chunk])
                nc.vector.tensor_mul(en[:], e16[:], r16[:])
                psW = pp.tile([128, m], F32, name="psW", tag="W",
                              padded_shape=[128, leaf_chunk])
                nc.tensor.matmul(psW[:], hext_sb[:], en[:], start=True, stop=True)
                hn = wk.tile([128, m], F32, name="hn", tag="hn",
                             padded_shape=[128, leaf_chunk])
                nc.vector.scalar_tensor_tensor(hn[:], hh2, 0.5, psW[:],
                                               OP.mult, OP.subtract)
                hout = h_st[lvl][:, c0:c0 + m]
                nc.vector.tensor_scalar_add(hout, hn[:], bv_sb[:, 7:8])
                nc.sync.dma_start(houts[lvl].ap()[:, c0:c0 + m], hout)

            # ---- leaf level ----
            n = levels[0]
            step = min(leaf_chunk, n)
            for c0 in range(0, n, step):
                m = min(step, n - c0)
                xt = xp.tile([128, m], F32, name="xt", tag="xt",
                             padded_shape=[128, leaf_chunk])
                nc.sync.dma_start(xt[:], xT.ap()[:, c0:c0 + m])
                psI = pp.tile([128, m], F32, name="psI", tag="I",
                              padded_shape=[128, leaf_chunk])
                psO = pp.tile([128, m], F32, name="psO", tag="O",
                              padded_shape=[128, leaf_chunk])
                psU = pp.tile([128, m], F32, name="psU", tag="U",
                              padded_shape=[128, leaf_chunk])
                nc.tensor.matmul(psI[:], wx_sb[:, 0:128], xt[:], start=True, stop=True)
                nc.tensor.matmul(psO[:], wx_sb[:, 128:256], xt[:], start=True, stop=True)
                nc.tensor.matmul(psU[:], wx_sb[:, 256:384], xt[:], start=True, stop=True)
                ti = wk.tile([128, m], F32, name="ti", tag="ti",
                             padded_shape=[128, leaf_chunk])
                to = wk.tile([128, m], F32, name="to", tag="to",
                             padded_shape=[128, leaf_chunk])
                tu = wk.tile([128, m], F32, name="tu", tag="tu",
                             padded_shape=[128, leaf_chunk])
                nc.scalar.activation(ti[:], psI[:], AF.Tanh, bias=bv_sb[:, 0:1])
                nc.scalar.activation(to[:], psO[:], AF.Tanh, bias=bv_sb[:, 1:2])
                nc.scalar.activation(tu[:], psU[:], AF.Tanh, bias=bv_sb[:, 2:3])
                # c2 = 2c = (ti + 1) * tu
                c2c = c2_st[0][:, c0:c0 + m]
                nc.vector.scalar_tensor_tensor(c2c, ti[:], 1.0, tu[:],
                                               OP.add, OP.mult)
                tcv = wk.tile([128, m], F32, name="tcv", tag="tcv",
                              padded_shape=[128, leaf_chunk])
                nc.scalar.activation(tcv[:], c2c, AF.Tanh, scale=0.5)
                hh2 = wk.tile([128, m], F32, name="hh2", tag="hh2",
                              padded_shape=[128, leaf_chunk])
                nc.vector.scalar_tensor_tensor(hh2[:], to[:], 1.0, tcv[:],
                                               OP.add, OP.mult)
                attention(hh2[:], m, 0, c0)

            # ---- internal levels ----
            for lvl in range(1, len(levels)):
                n = levels[lvl]
                hC = h_st[lvl - 1]
                c2C = c2_st[lvl - 1]
                step = min(int_chunk, n)
                for c0 in range(0, n, step):
                    m = min(step, n - c0)
                    ch0 = 2 * c0
                    # children slices (interleaved pairs)
                    hC_e = hC[:, ch0:ch0 + 2 * m:2]
                    hC_o = hC[:, ch0 + 1:ch0 + 2 * m:2]
                    c2_e = c2C[:, ch0:ch0 + 2 * m:2]
                    c2_o = c2C[:, ch0 + 1:ch0 + 2 * m:2]
                    csum2 = wk.tile([128, m], F32, name="csum2", tag="csum2",
                                    padded_shape=[128, int_chunk])
                    nc.vector.tensor_add(csum2[:], c2_e, c2_o)
                    psI = pp.tile([128, m], F32, name="psI", tag="I",
                                  padded_shape=[128, leaf_chunk])
                    psO = pp.tile([128, m], F32, name="psO", tag="O",
                                  padded_shape=[128, leaf_chunk])
                    psU = pp.tile([128, m], F32, name="psU", tag="U",
                                  padded_shape=[128, leaf_chunk])
                    # f-gate: children processed in halves of <=512 so psF
                    # stays one PSUM bank even with 512-wide parent chunks
                    Asum = wk.tile([128, m], F32, name="Asum", tag="Asum",
                                   padded_shape=[128, int_chunk])
                    nh = (2 * m + 511) // 512
                    hw_ = 2 * m // nh
                    first = True
                    for h0 in range(0, 2 * m, hw_):
                        psF = pp.tile([128, hw_], F32, name="psF", tag="F",
                                      padded_shape=[128, min(2 * int_chunk, 512)])
                        nc.tensor.matmul(psF[:], wf_sb[:],
                                         hC[:, ch0 + h0:ch0 + h0 + hw_],
                                         start=True, stop=True)
                        if first:
                            # iou on child-sum via PSUM accumulation of the
                            # even/odd child columns (no hsum DVE op needed)
                            for ps, w0 in ((psI, 0), (psO, 128), (psU, 256)):
                                nc.tensor.matmul(ps[:], wh_sb[:, w0:w0 + 128],
                                                 hC_e, start=True, stop=False)
                                nc.tensor.matmul(ps[:], wh_sb[:, w0:w0 + 128],
                                                 hC_o, start=False, stop=True)
                            first = False
                        tf = wk.tile([128, hw_], F32, name="tf", tag="tf",
                                     padded_shape=[128, min(2 * int_chunk, 512)])
                        nc.scalar.activation(tf[:], psF[:], AF.Tanh, bias=bv_sb[:, 6:7])
                        tfc = wk.tile([128, hw_], F32, name="tfc", tag="tfc",
                                      padded_shape=[128, min(2 * int_chunk, 512)])
                        nc.vector.tensor_mul(tfc[:], tf[:],
                                             c2C[:, ch0 + h0:ch0 + h0 + hw_])
                        nc.vector.tensor_add(Asum[:, h0 // 2:(h0 + hw_) // 2],
                                             tfc[:, 0:hw_:2], tfc[:, 1:hw_:2])
                    ti = wk.tile([128, m], F32, name="ti", tag="ti")
                    to = wk.tile([128, m], F32, name="to", tag="to")
                    tu = wk.tile([128, m], F32, name="tu", tag="tu")
                    nc.scalar.activation(ti[:], psI[:], AF.Tanh, bias=bv_sb[:, 3:4])
                    nc.scalar.activation(to[:], psO[:], AF.Tanh, bias=bv_sb[:, 4:5])
                    nc.scalar.activation(tu[:], psU[:], AF.Tanh, bias=bv_sb[:, 5:6])
                    p2 = wk.tile([128, m], F32, name="p2", tag="p2",
                                 padded_shape=[128, int_chunk])
                    nc.vector.scalar_tensor_tensor(p2[:], ti[:], 1.0, tu[:],
                                                   OP.add, OP.mult)
                    Dsum = wk.tile([128, m], F32, name="Dsum", tag="Dsum",
                                   padded_shape=[128, int_chunk])
                    nc.vector.tensor_add(Dsum[:], csum2[:], Asum[:])
                    # c2_new = p2 + 0.5 * (csum2 + Asum)
                    c2c = c2_st[lvl][:, c0:c0 + m]
                    nc.vector.scalar_tensor_tensor(c2c, Dsum[:], 0.5, p2[:],
                                                   OP.mult, OP.add)
                    tcv = wk.tile([128, m], F32, name="tcv", tag="tcv")
                    nc.scalar.activation(tcv[:], c2c, AF.Tanh, scale=0.5)
                    hh2 = wk.tile([128, m], F32, name="hh2", tag="hh2")
                    nc.vector.scalar_tensor_tensor(hh2[:], to[:], 1.0, tcv[:],
                                                   OP.add, OP.mult)
                    attention(hh2[:], m, lvl, c0)

            # last-level c output (true c = 0.5 * c2)
            cfin = wk.tile([128, levels[-1]], F32, name="cfin", tag="cfin")
            nc.vector.tensor_scalar_mul(cfin[:], c2_st[-1][:], 0.5)
            nc.sync.dma_start(clast.ap()[:], cfin[:])

    nc.compile()
    return nc, levels


def _get(leaf=LEAF, leaf_chunk=512, int_chunk=512):
    key = (leaf, leaf_chunk, int_chunk)
    if key not in _CACHE:
        _CACHE[key] = _build(leaf, leaf_chunk, int_chunk)
    return _CACHE[key]


def _np_sigmoid(x):
    return 1.0 / (1.0 + np.exp(-x))


def _np_level(c, h, Wiouh, biouh, Wfh, bfh):
    mem = Wiouh.shape[0]
    cc = c.reshape(-1, 2, mem)
    ch = h.reshape(-1, 2, mem)
    iou = ch.sum(axis=1) @ Wiouh + biouh
    i, o, u = np.split(iou, 3, axis=-1)
    f = _np_sigmoid(ch @ Wfh + bfh)
    c_new = _np_sigmoid(i) * np.tanh(u) + (f * cc).sum(axis=1)
    h_pre = _np_sigmoid(o) * np.tanh(c_new)
    return c_new, h_pre


def _np_attend(h, h_ext, Wattnh, battnh, Wa):
    n, d = h.shape
    ha = h @ Wattnh[:d, :]
    hb = h_ext @ Wattnh[d:, :] + battnh
    t = np.tanh(ha[:, None, :] + hb[None, :, :])
    z = t @ Wa
    z = z - z.max(axis=-1, keepdims=True)
    e = np.exp(z)
    s = e / e.sum(axis=-1, keepdims=True)
    return (1.0 - s) @ h_ext + s.sum(-1, keepdims=True) * h


def _preprocess(x, h_ext, Wioux, bioux, Wiouh, biouh, Wfh, bfh, Wattnh, battnh, Wa):
    f32 = np.float32
    Wx = np.array(Wioux, f32, copy=True)
    Wx[:, 0:128] *= 0.5
    Wx[:, 128:256] *= 0.5
    Wh = np.array(Wiouh, f32, copy=True)
    Wh[:, 0:128] *= 0.5
    Wh[:, 128:256] *= 0.5
    bl = np.asarray(bioux, f32) + np.asarray(biouh, f32)
    bi = np.asarray(biouh, f32)
    BV = np.stack(
        [
            0.5 * bl[0:128], 0.5 * bl[128:256], bl[256:384],
            0.5 * bi[0:128], 0.5 * bi[128:256], bi[256:384],
            0.5 * np.asarray(bfh, f32),
            np.asarray(h_ext, f32).sum(axis=0),
        ],
        axis=1,
    )
    Wf2 = 0.5 * np.asarray(Wfh, f32)
    W1h = np.ascontiguousarray(0.5 * np.asarray(Wattnh, f32)[:128, :])
    GBm = np.ascontiguousarray(
        (np.asarray(h_ext, f32) @ np.asarray(Wattnh, f32)[128:, :]
         + np.asarray(battnh, f32)).T
    )
    WAOH = np.zeros((128, 256), f32)
    for mm in range(16):
        WAOH[:, 16 * mm + mm] = np.asarray(Wa, f32)
    ONES = np.ones((16, 16), f32)
    HEXT = np.ascontiguousarray(np.asarray(h_ext, f32))
    F16m = np.exp(2.0 * GBm).astype(f32)
    wsum = float(np.asarray(Wa, np.float64).sum())
    SVm = np.zeros((16, 2), f32)
    SVm[:16 - KDVE, 0] = 1.0
    SVm[16 - KDVE:, 0] = -2.0
    SVm[16 - KDVE:, 1] = wsum
    return dict(Wx=Wx, Wh=Wh, Wf=Wf2, W1=W1h, GB=GBm, WAOH=WAOH, ONES=ONES,
                HEXT=HEXT, BV=np.ascontiguousarray(BV), F16=F16m, SV=SVm)


def kernel(x, h_ext, Wioux, bioux, Wiouh, biouh, Wfh, bfh, Wattnh, battnh, Wa,
           _run_device=None):
    f32 = np.float32
    x = np.asarray(x, f32)
    args = (x, np.asarray(h_ext, f32), np.asarray(Wioux, f32),
            np.asarray(bioux, f32), np.asarray(Wiouh, f32),
            np.asarray(biouh, f32), np.asarray(Wfh, f32), np.asarray(bfh, f32),
            np.asarray(Wattnh, f32), np.asarray(battnh, f32), np.asarray(Wa, f32))
    consts = _preprocess(*args)

    nc, levels = _get()
    in_maps = []
    for k in range(CORES):
        shard = np.ascontiguousarray(x[k * LEAF:(k + 1) * LEAF, :].T)
        in_maps.append({"xT": shard, **consts})

    if _run_device is None:
        res = bass_utils.run_bass_kernel_spmd(nc, in_maps, core_ids=list(range(CORES)))
        core_outs = res.results
    else:
        core_outs = _run_device(nc, in_maps)

    # ---- gather device outputs ----
    (_, h_ext_a, _, _, Wiouh_a, biouh_a, Wfh_a, bfh_a, Wattnh_a, battnh_a,
     Wa_a) = args
    n_levels = len(levels)
    # full per-level h arrays, node-major
    full_h = []
    for i, nl in enumerate(levels):
        arr = np.empty((CORES * nl, MEM), f32)
        for k in range(CORES):
            arr[k * nl:(k + 1) * nl] = core_outs[k][f"h{i}"].T
        full_h.append(arr)
    nlast = levels[-1]
    c8 = np.concatenate([core_outs[k]["c_last"].T for k in range(CORES)], axis=0)
    h8 = full_h[-1]

    # ---- host: finish top levels ----
    c, h = c8, h8
    host_h = []
    while c.shape[0] > 2:
        c, hpre = _np_level(c, h, Wiouh_a, biouh_a, Wfh_a, bfh_a)
        h = _np_attend(hpre, h_ext_a, Wattnh_a, battnh_a, Wa_a)
        host_h.append(h)

    out = np.concatenate([c, h] + full_h + host_h, axis=0)
    return out.astype(f32)


if __name__ == "__main__":
    import reference

    inputs = {k: np.asarray(v) for k, v in reference.setup_inputs().items()}
    out = kernel(**inputs)
    print(out.shape, out.dtype)



# revision 10
# speedup vs baseline: 3.7261x; 3.7261x over previous
"""ChildSumTreeLSTM (complete binary tree, L=16384 leaves, mem=128) on 8 NeuronCores.

Sharding: 8 subtrees of 2048 leaves, data-parallel per level. Each core runs the
same Bass/Tile program on its shard, computing its subtree's 12 levels
(2048 -> 1 nodes). The final two tree levels (4 + 2 nodes) are finished on host.

Layout: feature-major (mem dim on SBUF partitions, nodes along free dim) so all
matmuls need no on-device transposes and the attention bias (h_ext @ W2 + b) is
a per-partition ACT bias fused into the tanh.

Numerics: sigmoid(x) = 0.5 + 0.5*tanh(x/2) folded into pre-scaled weights, so
the whole kernel uses only {tanh, exp} (one ACT table set, loaded once) plus a
DVE reciprocal approximation for the softmax normalization.
"""

import numpy as np

try:
    import concourse.bass as bass
except ImportError:
    import sys

    for p in ("/opt/trn_rl_repo", "/root/.axon_site/_ro/trn_rl_repo"):
        if p not in sys.path:
            sys.path.insert(0, p)
    import concourse.bass as bass

import concourse.bacc as bacc
import concourse.mybir as mybir
import concourse.tile as tile
from concourse import bass_utils

F32 = mybir.dt.float32
AF = mybir.ActivationFunctionType
OP = mybir.AluOpType

L = 16384
MEM = 128
CORES = 8
KDVE = 7  # m-streams routed via DVE exp-path at big levels
LEAF = L // CORES  # 2048 leaves per core

_CACHE = {}

# packed-const layout: name -> (col offset, width, partitions)
_COFF = {
    "Wx": (0, 384, 128),
    "Wh": (384, 384, 128),
    "Wf": (768, 128, 128),
    "W1": (896, 128, 128),
    "GB": (1024, 16, 128),
    "WAOH": (1040, 256, 128),
    "BV": (1296, 8, 128),
    "F16": (1304, 16, 128),
    "ONES": (1320, 16, 16),
    "HEXT": (1336, 128, 16),
    "SV": (1464, 2, 16),
}
_CW = 1466  # total packed width


def _build(leaf=LEAF, leaf_chunk=512, int_chunk=512, attn=True, min_n=4):
    """Build + compile the per-core Bass program. Returns (nc, levels)."""
    levels = []
    n = leaf
    while n >= min_n and n >= leaf // 512:
        levels.append(n)
        n //= 2

    nc = bacc.Bacc("TRN2", debug=False)

    xT = nc.dram_tensor("xT", [128, leaf], F32, kind="ExternalInput")
    CONST = nc.dram_tensor("CONST", [128, _CW], F32, kind="ExternalInput")
    # packed output: h for every level, then c of the last level
    lvl_off = []
    off = 0
    for nl in levels:
        lvl_off.append(off)
        off += nl
    c_off = off
    off += levels[-1]
    OUT = nc.dram_tensor("OUT", [128, off], F32, kind="ExternalOutput")

    with tile.TileContext(nc) as tc:
        with (
            tc.tile_pool(name="const", bufs=1) as cp,
            tc.tile_pool(name="state", bufs=1) as st,
            tc.tile_pool(name="work", bufs=3) as wk,
            tc.tile_pool(name="xin", bufs=3) as xp,
            tc.tile_pool(name="psum", bufs=1, space="PSUM") as pp,
        ):
            # ---- constants into SBUF (one DMA, sliced views) ----
            call = cp.tile([128, _CW], F32)
            nc.sync.dma_start(call[:], CONST.ap())

            def cview(name):
                o, w, p = _COFF[name]
                return call[0:p, o:o + w]

            wx_sb = cview("Wx")
            wh_sb = cview("Wh")
            wf_sb = cview("Wf")
            w1_sb = cview("W1")
            gb_sb = cview("GB")
            waoh_sb = cview("WAOH")
            ones_sb = cview("ONES")
            hext_sb = cview("HEXT")
            bv_sb = cview("BV")
            f16_sb = cview("F16")
            sv_sb = cview("SV")

            # per-level persistent state: h (true post-attention h) and c2 = 2*c
            h_st = [st.tile([128, nl], F32, name=f"hst{i}", tag=f"hst{i}")
                    for i, nl in enumerate(levels)]
            c2_st = [st.tile([128, nl], F32, name=f"cst{i}", tag=f"cst{i}")
                     for i, nl in enumerate(levels)]

            def attention(hh2, m, lvl, c0):
                """hh2 = 2*h_pre [128, m]; writes h_st[lvl][:, c0:c0+m] and DMAs out."""
                if not attn:
                    hout = h_st[lvl][:, c0:c0 + m]
                    nc.vector.tensor_scalar_mul(hout, hh2, 0.5)
                    nc.sync.dma_start(
                        OUT.ap()[:, lvl_off[lvl] + c0:lvl_off[lvl] + c0 + m], hout)
                    return
                psH = pp.tile([128, m], F32, name="psH", tag="H",
                              padded_shape=[128, leaf_chunk])
                nc.tensor.matmul(psH[:], w1_sb[:], hh2, start=True, stop=True)
                psZ = pp.tile([16, m], F32, name="psZ", tag="Z",
                              padded_shape=[16, leaf_chunk])
                if 16 * m <= 512:
                    # deep levels: build all 16 tanh args with one broadcast
                    # add + one tanh, instead of 16 separate ACT ops
                    targ = wk.tile([128, 16 * m], F32, name="targ", tag="targ",
                                   padded_shape=[128, 512])
                    t3 = targ[:].rearrange("p (g j) -> p g j", g=16)
                    nc.vector.tensor_tensor(
                        t3,
                        psH[:].unsqueeze(1).to_broadcast((128, 16, m)),
                        gb_sb[:].unsqueeze(2).to_broadcast((128, 16, m)),
                        OP.add,
                    )
                    tall = wk.tile([128, 16 * m], F32, name="tall", tag="tall",
                                   padded_shape=[128, 512])
                    nc.scalar.activation(tall[:], targ[:], AF.Tanh)
                    for mm in range(16):
                        nc.tensor.matmul(psZ[:], waoh_sb[:, 16 * mm:16 * mm + 16],
                                         tall[:, mm * m:(mm + 1) * m],
                                         start=(mm == 0), stop=(mm == 15))
                else:
                    # split the 16 m-streams between ACT (tanh) and DVE
                    # (1/(e^{2ha}e^{2g}+1) via recip approx); the -2x+Wsum
                    # correction for DVE rows is folded into the softmax exp
                    # below via per-row scale/bias vectors.
                    ebuf = wk.tile([128, m], F32, name="ebuf", tag="ebuf",
                                   padded_shape=[128, leaf_chunk])
                    nc.scalar.activation(ebuf[:], psH[:], AF.Exp, scale=2.0)
                    for mm in range(16):
                        if mm < 16 - KDVE:
                            tm = wk.tile([128, m], F32, name="tm", tag="tm",
                                         bufs=3, padded_shape=[128, leaf_chunk])
                            nc.scalar.activation(tm[:], psH[:], AF.Tanh,
                                                 bias=gb_sb[:, mm:mm + 1])
                            rhs = tm
                        else:
                            gm = wk.tile([128, m], F32, name="gm", tag="gm",
                                         bufs=3, padded_shape=[128, leaf_chunk])
                            nc.vector.tensor_scalar(
                                gm[:], ebuf[:], f16_sb[:, mm:mm + 1], 1.0,
                                OP.mult, OP.add)
                            rm = wk.tile([128, m], F32, name="rm", tag="rm",
                                         bufs=3, padded_shape=[128, leaf_chunk])
                            nc.vector.reciprocal_approx_fast(rm[:], gm[:])
                            rhs = rm
                        nc.tensor.matmul(psZ[:], waoh_sb[:, 16 * mm:16 * mm + 16],
                                         rhs[:], start=(mm == 0), stop=(mm == 15))
                e16 = wk.tile([16, m], F32, name="e16", tag="e16",
                              padded_shape=[16, leaf_chunk])
                if 16 * m <= 512:
                    nc.scalar.activation(e16[:], psZ[:], AF.Exp)
                else:
                    nc.scalar.activation(e16[:], psZ[:], AF.Exp,
                                         bias=sv_sb[:, 1:2], scale=sv_sb[:, 0:1])
                psS = pp.tile([16, m], F32, name="psS", tag="S",
                              padded_shape=[16, leaf_chunk])
                nc.tensor.matmul(psS[:], ones_sb[:], e16[:], start=True, stop=True)
                r16 = wk.tile([16, m], F32, name="r16", tag="r16",
                              padded_shape=[16, leaf_chunk])
                nc.vector.reciprocal_approx_fast(r16[:], psS[:])
                en = wk.tile([16, m], F32, name="en", tag="en",
                             padded_shape=[16, leaf_chunk])
                nc.vector.tensor_mul(en[:], e16[:], r16[:])
                psW = pp.tile([128, m], F32, name="psW", tag="W",
                              padded_shape=[128, leaf_chunk])
                nc.tensor.matmul(psW[:], hext_sb[:], en[:], start=True, stop=True)
                hn = wk.tile([128, m], F32, name="hn", tag="hn",
                             padded_shape=[128, leaf_chunk])
                nc.vector.scalar_tensor_tensor(hn[:], hh2, 0.5, psW[:],
                                               OP.mult, OP.subtract)
                hout = h_st[lvl][:, c0:c0 + m]
                nc.vector.tensor_scalar_add(hout, hn[:], bv_sb[:, 7:8])
                nc.sync.dma_start(
                    OUT.ap()[:, lvl_off[lvl] + c0:lvl_off[lvl] + c0 + m], hout)

            # ---- leaf level ----
            n = levels[0]
            step = min(leaf_chunk, n)
            for c0 in range(0, n, step):
                m = min(step, n - c0)
                xt = xp.tile([128, m], F32, name="xt", tag="xt",
                             padded_shape=[128, leaf_chunk])
                nc.sync.dma_start(xt[:], xT.ap()[:, c0:c0 + m])
                psI = pp.tile([128, m], F32, name="psI", tag="I",
                              padded_shape=[128, leaf_chunk])
                psO = pp.tile([128, m], F32, name="psO", tag="O",
                              padded_shape=[128, leaf_chunk])
                psU = pp.tile([128, m], F32, name="psU", tag="U",
                              padded_shape=[128, leaf_chunk])
                nc.tensor.matmul(psI[:], wx_sb[:, 0:128], xt[:], start=True, stop=True)
                nc.tensor.matmul(psO[:], wx_sb[:, 128:256], xt[:], start=True, stop=True)
                nc.tensor.matmul(psU[:], wx_sb[:, 256:384], xt[:], start=True, stop=True)
                ti = wk.tile([128, m], F32, name="ti", tag="ti",
                             padded_shape=[128, leaf_chunk])
                to = wk.tile([128, m], F32, name="to", tag="to",
                             padded_shape=[128, leaf_chunk])
                tu = wk.tile([128, m], F32, name="tu", tag="tu",
                             padded_shape=[128, leaf_chunk])
                nc.scalar.activation(ti[:], psI[:], AF.Tanh, bias=bv_sb[:, 0:1])
                nc.scalar.activation(to[:], psO[:], AF.Tanh, bias=bv_sb[:, 1:2])
                nc.scalar.activation(tu[:], psU[:], AF.Tanh, bias=bv_sb[:, 2:3])
                # c2 = 2c = (ti + 1) * tu
                c2c = c2_st[0][:, c0:c0 + m]
                nc.vector.scalar_tensor_tensor(c2c, ti[:], 1.0, tu[:],
                                               OP.add, OP.mult)
                tcv = wk.tile([128, m], F32, name="tcv", tag="tcv",
                              padded_shape=[128, leaf_chunk])
                nc.scalar.activation(tcv[:], c2c, AF.Tanh, scale=0.5)
                hh2 = wk.tile([128, m], F32, name="hh2", tag="hh2",
                              padded_shape=[128, leaf_chunk])
                nc.vector.scalar_tensor_tensor(hh2[:], to[:], 1.0, tcv[:],
                                               OP.add, OP.mult)
                attention(hh2[:], m, 0, c0)

            # ---- internal levels ----
            for lvl in range(1, len(levels)):
                n = levels[lvl]
                hC = h_st[lvl - 1]
                c2C = c2_st[lvl - 1]
                step = min(int_chunk, n)
                for c0 in range(0, n, step):
                    m = min(step, n - c0)
                    ch0 = 2 * c0
                    # children slices (interleaved pairs)
                    hC_e = hC[:, ch0:ch0 + 2 * m:2]
                    hC_o = hC[:, ch0 + 1:ch0 + 2 * m:2]
                    c2_e = c2C[:, ch0:ch0 + 2 * m:2]
                    c2_o = c2C[:, ch0 + 1:ch0 + 2 * m:2]
                    csum2 = wk.tile([128, m], F32, name="csum2", tag="csum2",
                                    padded_shape=[128, int_chunk])
                    nc.vector.tensor_add(csum2[:], c2_e, c2_o)
                    psI = pp.tile([128, m], F32, name="psI", tag="I",
                                  padded_shape=[128, leaf_chunk])
                    psO = pp.tile([128, m], F32, name="psO", tag="O",
                                  padded_shape=[128, leaf_chunk])
                    psU = pp.tile([128, m], F32, name="psU", tag="U",
                                  padded_shape=[128, leaf_chunk])
                    # f-gate: children processed in halves of <=512 so psF
                    # stays one PSUM bank even with 512-wide parent chunks
                    Asum = wk.tile([128, m], F32, name="Asum", tag="Asum",
                                   padded_shape=[128, int_chunk])
                    nh = (2 * m + 511) // 512
                    hw_ = 2 * m // nh
                    first = True
                    for h0 in range(0, 2 * m, hw_):
                        psF = pp.tile([128, hw_], F32, name="psF", tag="F",
                                      padded_shape=[128, min(2 * int_chunk, 512)])
                        nc.tensor.matmul(psF[:], wf_sb[:],
                                         hC[:, ch0 + h0:ch0 + h0 + hw_],
                                         start=True, stop=True)
                        if first:
                            # iou on child-sum via PSUM accumulation of the
                            # even/odd child columns (no hsum DVE op needed)
                            for ps, w0 in ((psI, 0), (psO, 128), (psU, 256)):
                                nc.tensor.matmul(ps[:], wh_sb[:, w0:w0 + 128],
                                                 hC_e, start=True, stop=False)
                                nc.tensor.matmul(ps[:], wh_sb[:, w0:w0 + 128],
                                                 hC_o, start=False, stop=True)
                            first = False
                        tf = wk.tile([128, hw_], F32, name="tf", tag="tf",
                                     padded_shape=[128, min(2 * int_chunk, 512)])
                        nc.scalar.activation(tf[:], psF[:], AF.Tanh, bias=bv_sb[:, 6:7])
                        tfc = wk.tile([128, hw_], F32, name="tfc", tag="tfc",
                                      padded_shape=[128, min(2 * int_chunk, 512)])
                        nc.vector.tensor_mul(tfc[:], tf[:],
                                             c2C[:, ch0 + h0:ch0 + h0 + hw_])
                        nc.vector.tensor_add(Asum[:, h0 // 2:(h0 + hw_) // 2],
                                             tfc[:, 0:hw_:2], tfc[:, 1:hw_:2])
                    ti = wk.tile([128, m], F32, name="ti", tag="ti")
                    to = wk.tile([128, m], F32, name="to", tag="to")
                    tu = wk.tile([128, m], F32, name="tu", tag="tu")
                    nc.scalar.activation(ti[:], psI[:], AF.Tanh, bias=bv_sb[:, 3:4])
                    nc.scalar.activation(to[:], psO[:], AF.Tanh, bias=bv_sb[:, 4:5])
                    nc.scalar.activation(tu[:], psU[:], AF.Tanh, bias=bv_sb[:, 5:6])
                    p2 = wk.tile([128, m], F32, name="p2", tag="p2",
                                 padded_shape=[128, int_chunk])
                    nc.vector.scalar_tensor_tensor(p2[:], ti[:], 1.0, tu[:],
                                                   OP.add, OP.mult)
                    Dsum = wk.tile([128, m], F32, name="Dsum", tag="Dsum",
                                   padded_shape=[128, int_chunk])
                    nc.vector.tensor_add(Dsum[:], csum2[:], Asum[:])
                    # c2_new = p2 + 0.5 * (csum2 + Asum)
                    c2c = c2_st[lvl][:, c0:c0 + m]
                    nc.vector.scalar_tensor_tensor(c2c, Dsum[:], 0.5, p2[:],
                                                   OP.mult, OP.add)
                    tcv = wk.tile([128, m], F32, name="tcv", tag="tcv")
                    nc.scalar.activation(tcv[:], c2c, AF.Tanh, scale=0.5)
                    hh2 = wk.tile([128, m], F32, name="hh2", tag="hh2")
                    nc.vector.scalar_tensor_tensor(hh2[:], to[:], 1.0, tcv[:],
                                                   OP.add, OP.mult)
                    attention(hh2[:], m, lvl, c0)

            # last-level c output (true c = 0.5 * c2)
            cfin = wk.tile([128, levels[-1]], F32, name="cfin", tag="cfin")
            nc.vector.tensor_scalar_mul(cfin[:], c2_st[-1][:], 0.5)
            nc.sync.dma_start(OUT.ap()[:, c_off:c_off + levels[-1]], cfin[:])

    nc.compile()
    return nc, levels


def _get(leaf=LEAF, leaf_chunk=512, int_chunk=512, attn=True, min_n=4):
    key = (leaf, leaf_chunk, int_chunk, attn, min_n)
    if key not in _CACHE:
        _CACHE[key] = _build(leaf, leaf_chunk, int_chunk, attn, min_n)
    return _CACHE[key]


def _np_sigmoid(x):
    return 1.0 / (1.0 + np.exp(-x))


def _np_level(c, h, Wiouh, biouh, Wfh, bfh):
    mem = Wiouh.shape[0]
    cc = c.reshape(-1, 2, mem)
    ch = h.reshape(-1, 2, mem)
    iou = ch.sum(axis=1) @ Wiouh + biouh
    i, o, u = np.split(iou, 3, axis=-1)
    f = _np_sigmoid(ch @ Wfh + bfh)
    c_new = _np_sigmoid(i) * np.tanh(u) + (f * cc).sum(axis=1)
    h_pre = _np_sigmoid(o) * np.tanh(c_new)
    return c_new, h_pre


def _np_attend(h, h_ext, Wattnh, battnh, Wa):
    n, d = h.shape
    ha = h @ Wattnh[:d, :]
    hb = h_ext @ Wattnh[d:, :] + battnh
    t = np.tanh(ha[:, None, :] + hb[None, :, :])
    z = t @ Wa
    z = z - z.max(axis=-1, keepdims=True)
    e = np.exp(z)
    s = e / e.sum(axis=-1, keepdims=True)
    return (1.0 - s) @ h_ext + s.sum(-1, keepdims=True) * h


def _preprocess(x, h_ext, Wioux, bioux, Wiouh, biouh, Wfh, bfh, Wattnh, battnh, Wa):
    f32 = np.float32
    Wx = np.array(Wioux, f32, copy=True)
    Wx[:, 0:128] *= 0.5
    Wx[:, 128:256] *= 0.5
    Wh = np.array(Wiouh, f32, copy=True)
    Wh[:, 0:128] *= 0.5
    Wh[:, 128:256] *= 0.5
    bl = np.asarray(bioux, f32) + np.asarray(biouh, f32)
    bi = np.asarray(biouh, f32)
    BV = np.stack(
        [
            0.5 * bl[0:128], 0.5 * bl[128:256], bl[256:384],
            0.5 * bi[0:128], 0.5 * bi[128:256], bi[256:384],
            0.5 * np.asarray(bfh, f32),
            np.asarray(h_ext, f32).sum(axis=0),
        ],
        axis=1,
    )
    Wf2 = 0.5 * np.asarray(Wfh, f32)
    W1h = np.ascontiguousarray(0.5 * np.asarray(Wattnh, f32)[:128, :])
    GBm = np.ascontiguousarray(
        (np.asarray(h_ext, f32) @ np.asarray(Wattnh, f32)[128:, :]
         + np.asarray(battnh, f32)).T
    )
    WAOH = np.zeros((128, 256), f32)
    for mm in range(16):
        WAOH[:, 16 * mm + mm] = np.asarray(Wa, f32)
    ONES = np.ones((16, 16), f32)
    HEXT = np.ascontiguousarray(np.asarray(h_ext, f32))
    F16m = np.exp(2.0 * GBm).astype(f32)
    wsum = float(np.asarray(Wa, np.float64).sum())
    SVm = np.zeros((16, 2), f32)
    SVm[:16 - KDVE, 0] = 1.0
    SVm[16 - KDVE:, 0] = -2.0
    SVm[16 - KDVE:, 1] = wsum
    vals = dict(Wx=Wx, Wh=Wh, Wf=Wf2, W1=W1h, GB=GBm, WAOH=WAOH, ONES=ONES,
                HEXT=HEXT, BV=np.ascontiguousarray(BV), F16=F16m, SV=SVm)
    packed = np.zeros((128, _CW), f32)
    for name, (o, w, p) in _COFF.items():
        packed[0:p, o:o + w] = vals[name]
    return packed


def kernel(x, h_ext, Wioux, bioux, Wiouh, biouh, Wfh, bfh, Wattnh, battnh, Wa,
           _run_device=None):
    f32 = np.float32
    x = np.asarray(x, f32)
    args = (x, np.asarray(h_ext, f32), np.asarray(Wioux, f32),
            np.asarray(bioux, f32), np.asarray(Wiouh, f32),
            np.asarray(biouh, f32), np.asarray(Wfh, f32), np.asarray(bfh, f32),
            np.asarray(Wattnh, f32), np.asarray(battnh, f32), np.asarray(Wa, f32))
    consts = _preprocess(*args)

    nc, levels = _get()
    in_maps = []
    for k in range(CORES):
        shard = np.ascontiguousarray(x[k * LEAF:(k + 1) * LEAF, :].T)
        in_maps.append({"xT": shard, "CONST": consts})

    if _run_device is None:
        res = bass_utils.run_bass_kernel_spmd(nc, in_maps, core_ids=list(range(CORES)))
        core_outs = res.results
    else:
        core_outs = _run_device(nc, in_maps)

    # ---- gather device outputs ----
    (_, h_ext_a, _, _, Wiouh_a, biouh_a, Wfh_a, bfh_a, Wattnh_a, battnh_a,
     Wa_a) = args
    lvl_off = []
    off = 0
    for nl in levels:
        lvl_off.append(off)
        off += nl
    c_off = off
    # full per-level h arrays, node-major
    full_h = []
    for i, nl in enumerate(levels):
        arr = np.empty((CORES * nl, MEM), f32)
        for k in range(CORES):
            arr[k * nl:(k + 1) * nl] = core_outs[k]["OUT"][:, lvl_off[i]:lvl_off[i] + nl].T
        full_h.append(arr)
    nlast = levels[-1]
    c8 = np.concatenate(
        [core_outs[k]["OUT"][:, c_off:c_off + nlast].T for k in range(CORES)], axis=0)
    h8 = full_h[-1]

    # ---- host: finish top levels ----
    c, h = c8, h8
    host_h = []
    while c.shape[0] > 2:
        c, hpre = _np_level(c, h, Wiouh_a, biouh_a, Wfh_a, bfh_a)
        h = _np_attend(hpre, h_ext_a, Wattnh_a, battnh_a, Wa_a)
        host_h.append(h)

    out = np.concatenate([c, h] + full_h + host_h, axis=0)
    return out.astype(f32)


if __name__ == "__main__":
    import reference

    inputs = {k: np.asarray(v) for k, v in reference.setup_inputs().items()}
    out = kernel(**inputs)
    print(out.shape, out.dtype)

